# revision 3
# baseline (speedup 1.0000x reference)
"""Trainium2 Bass kernel for nn_DenseSOFLayer (diag-Gaussian log-prob, GEMM form).

out[b, f] = -0.5 * sum_d ((x[b,d] - mu[f,d]) / scale[f,d])^2
          = sum_d x^2[b,d] * w1[f,d] + x[b,d] * w2[f,d] + mm[f]
  w1 = -1/(2 s^2), w2 = mu/s^2, mm[f] = sum_d -mu^2/(2 s^2)

Sharding: 2 (batch) x 4 (feature) grid over 8 cores.
Per core: C_local[4096, 1024] = A[4096, 2048] @ W[2048, 1024] via f32r matmuls
(full-rate PE, ~12-bit mantissa) with the mm row folded in during PSUM
evacuation through a DMA partition-broadcast.
"""

import sys

if "/opt/trn_rl_repo" not in sys.path:
    sys.path.insert(0, "/opt/trn_rl_repo")

import numpy as np

import concourse.bass as bass
import concourse.mybir as mybir
import concourse.tile as tile
from concourse import bacc, bass_utils

f32 = mybir.dt.float32
f32r = mybir.dt.float32r

B, F, D = 8192, 4096, 1024
NB, NF = 2, 4              # core grid: batch-split x feature-split
BL, FL = B // NB, F // NF  # 4096, 1024 per core
MT = BL // 128             # 32 m-tiles
NT = FL // 512             # 2 n-tiles
KD = D // 128              # 8 contraction tiles per GEMM half

_cache = {}


def build_nc():
    """Build + compile the per-core Bass program (cached)."""
    if "nc" in _cache:
        return _cache["nc"]

    nc = bacc.Bacc("TRN2", target_bir_lowering=False, debug=False)
    # x.T pre-tiled on host: xt[m, p, k, j] = x[m*128+j, k*128+p]
    # -> per (m): SBUF tile [128, 1024] with fully-contiguous 4KB per partition
    xt_d = nc.dram_tensor("xt", [MT, 128, D], f32r, kind="ExternalInput").ap()
    mut_d = nc.dram_tensor("mut", [D, FL], f32, kind="ExternalInput").ap()
    sct_d = nc.dram_tensor("sct", [D, FL], f32, kind="ExternalInput").ap()
    out_d = nc.dram_tensor("out", [BL, FL], f32, kind="ExternalOutput").ap()

    with tile.TileContext(nc) as tc:
        with (
            nc.allow_low_precision(reason="f32r GEMM: ~12-bit mantissa is within the accuracy budget"),
            tc.tile_pool(name="wpool", bufs=1) as wpool,
            tc.tile_pool(name="stage", bufs=3) as stage,
            tc.tile_pool(name="cpool", bufs=1) as cpool,
            tc.tile_pool(name="dram", bufs=1, space="DRAM") as dram,
            tc.tile_pool(name="xpool", bufs=3) as xpool,
            tc.tile_pool(name="opool", bufs=3) as opool,
            tc.tile_pool(name="pspool", bufs=4, space="PSUM") as pspool,
            tc.tile_pool(name="mmps", bufs=1, space="PSUM") as mmps_pool,
        ):
            # ones column (f32r) for the partition-dim reduction of mm
            ones_t = cpool.tile([128, 1], f32, tag="ones")
            nc.gpsimd.memset(ones_t[:], 1.0)
            ones_r = cpool.tile([128, 1], f32r, tag="ones_r")
            nc.vector.tensor_copy(ones_r[:], ones_t[:])

            # ---- prologue: W = f(mu, scale) tiles, resident in SBUF ----
            w1 = {}
            w2 = {}
            mmps = [
                mmps_pool.tile([1, 512], f32, tag=f"mmps{n}", name=f"mmps{n}")
                for n in range(NT)
            ]
            for k in range(KD):
                for n in range(NT):
                    ksl = slice(k * 128, (k + 1) * 128)
                    nsl = slice(n * 512, (n + 1) * 512)
                    mt_t = stage.tile([128, 512], f32, tag="mt")
                    nc.sync.dma_start(mt_t[:], mut_d[ksl, nsl])
                    st_t = stage.tile([128, 512], f32, tag="st")
                    nc.sync.dma_start(st_t[:], sct_d[ksl, nsl])
                    s2 = stage.tile([128, 512], f32, tag="s2")
                    nc.vector.tensor_mul(s2[:], st_t[:], st_t[:])
                    nc.vector.tensor_scalar_mul(s2[:], s2[:], -2.0)
                    w1t = wpool.tile([128, 512], f32r, tag=f"w1_{k}_{n}")
                    nc.vector.reciprocal(w1t[:], s2[:])  # -1/(2 s^2)
                    inv = stage.tile([128, 512], f32, tag="inv")
                    nc.vector.tensor_scalar_mul(inv[:], w1t[:].bitcast(f32), -2.0)
                    w2t = wpool.tile([128, 512], f32r, tag=f"w2_{k}_{n}")
                    nc.vector.tensor_mul(w2t[:], mt_t[:], inv[:])  # mu/s^2
                    mu2 = stage.tile([128, 512], f32, tag="mu2")
                    nc.vector.tensor_mul(mu2[:], mt_t[:], mt_t[:])
                    m2i = stage.tile([128, 512], f32r, tag="m2i")
                    nc.vector.tensor_mul(m2i[:], mu2[:], w1t[:].bitcast(f32))
                    nc.tensor.matmul(
                        mmps[n][:], ones_r[:], m2i[:],
                        start=(k == 0), stop=(k == KD - 1),
                    )
                    w1[(k, n)] = w1t
                    w2[(k, n)] = w2t

            # mm row -> DRAM -> partition-broadcast back to all 128 partitions
            mmsb = cpool.tile([1, FL], f32, tag="mmsb")
            for n in range(NT):
                nc.vector.tensor_copy(mmsb[:, n * 512:(n + 1) * 512], mmps[n][:])
            mm_dram = dram.tile([1, FL], f32)
            nc.sync.dma_start(mm_dram[:], mmsb[:])
            mmbc = cpool.tile([128, FL], f32, tag="mmbc")
            nc.sync.dma_start(mmbc[:], mm_dram[:].to_broadcast((128, FL)))

            # ---- main loop: C[m] = x2 @ w1 + x @ w2 + mm ----
            for m in range(MT):
                xs = xpool.tile([128, D], f32r, tag="xs")
                nc.sync.dma_start(xs[:], xt_d[m])
                xq = xpool.tile([128, D], f32r, tag="xq")
                nc.vector.tensor_mul(xq[:], xs[:].bitcast(f32), xs[:].bitcast(f32))
                ot = opool.tile([128, FL], f32, tag="ot")
                for n in range(NT):
                    ps = pspool.tile([128, 512], f32, tag="ps")
                    for k in range(KD):
                        nc.tensor.matmul(
                            ps[:], xq[:, k * 128:(k + 1) * 128], w1[(k, n)][:],
                            start=(k == 0), stop=False,
                        )
                    for k in range(KD):
                        nc.tensor.matmul(
                            ps[:], xs[:, k * 128:(k + 1) * 128], w2[(k, n)][:],
                            start=False, stop=(k == KD - 1),
                        )
                    nc.vector.tensor_add(
                        ot[:, n * 512:(n + 1) * 512], ps[:],
                        mmbc[:, n * 512:(n + 1) * 512],
                    )
                nc.sync.dma_start(out_d[m * 128:(m + 1) * 128, :], ot[:])

    nc.compile()
    _cache["nc"] = nc
    return nc


def make_in_maps(x, mu, scale_diag):
    """Host-side shard + layout prep (free: not on the measured HW path)."""
    x = np.ascontiguousarray(x, dtype=np.float32)
    mu = np.ascontiguousarray(mu, dtype=np.float32)
    scale_diag = np.ascontiguousarray(scale_diag, dtype=np.float32)

    in_maps = []
    for c in range(NB * NF):
        ib, jf = divmod(c, NF)
        xsl = x[ib * BL:(ib + 1) * BL]  # [4096, 1024]
        # xt[m, p, k*128+j] = xsl[m*128+j, k*128+p]
        xt = np.ascontiguousarray(
            xsl.reshape(MT, 128, KD, 128).transpose(0, 3, 2, 1).reshape(MT, 128, D)
        )
        musl = mu[jf * FL:(jf + 1) * FL]        # [1024, 1024]
        scsl = scale_diag[jf * FL:(jf + 1) * FL]
        in_maps.append({
            "xt": xt,
            "mut": np.ascontiguousarray(musl.T),
            "sct": np.ascontiguousarray(scsl.T),
        })
    return in_maps


def gather(results):
    out = np.empty((B, F), dtype=np.float32)
    for c in range(NB * NF):
        ib, jf = divmod(c, NF)
        out[ib * BL:(ib + 1) * BL, jf * FL:(jf + 1) * FL] = results[c]["out"]
    return out


def kernel(x, mu, scale_diag):
    nc = build_nc()
    in_maps = make_in_maps(x, mu, scale_diag)
    r = bass_utils.run_bass_kernel_spmd(nc, in_maps, core_ids=list(range(NB * NF)))
    return gather(r.results)


if __name__ == "__main__":
    rng = np.random.default_rng(0)
    x = rng.standard_normal((B, D), dtype=np.float32)
    mu = rng.standard_normal((F, D), dtype=np.float32)
    sc = rng.uniform(0.5, 1.5, size=(F, D)).astype(np.float32)
    got = kernel(x, mu, sc)
    inv2 = 1.0 / (sc.astype(np.float64) ** 2)
    xx = (x.astype(np.float64) ** 2) @ inv2.T
    xm = x.astype(np.float64) @ (mu * inv2).T
    mm = (mu.astype(np.float64) ** 2 * inv2).sum(-1)
    want = -0.5 * (xx - 2 * xm + mm[None, :])
    err = np.abs(got - want).max() / np.abs(want).max()
    print("rel err vs fp64:", err)


# revision 10
# speedup vs baseline: 56.7335x; 56.7335x over previous
"""Trainium2 Bass kernel for nn_DenseSOFLayer (diag-Gaussian log-prob, GEMM form).

out[b, f] = -0.5 * sum_d ((x[b,d] - mu[f,d]) / scale[f,d])^2
          = sum_d x^2[b,d] * w1[f,d] + x[b,d] * w2[f,d] + mm[f]
  w1 = -1/(2 s^2), w2 = mu/s^2, mm[f] = sum_d mu^2 * w1

Sharding: 2 (batch) x 4 (feature) grid over 8 cores.
Per core: C_local[4096, 1024] = A[4096, 2048] @ W[2048, 1024] via f32r matmuls
(full-rate PE, ~12-bit mantissa) with the mm row folded in during PSUM
evacuation through a DMA partition-broadcast.

Emission order is tuned so the PE starts the m=0/m=1 output tiles while the
W-prep (ACT+DVE) is still streaming k-tiles, instead of idling behind the
whole prologue.
"""

import sys

if "/opt/trn_rl_repo" not in sys.path:
    sys.path.insert(0, "/opt/trn_rl_repo")

import numpy as np

import concourse.bass as bass
import concourse.mybir as mybir
import concourse.tile as tile
from concourse import bacc, bass_utils

f32 = mybir.dt.float32
f32r = mybir.dt.float32r
ACTF = mybir.ActivationFunctionType

B, F, D = 8192, 4096, 1024
NB, NF = 2, 4              # core grid: batch-split x feature-split
BL, FL = B // NB, F // NF  # 4096, 1024 per core
MT = BL // 128             # 32 m-tiles
NT = FL // 512             # 2 n-tiles
KD = D // 128              # 8 contraction tiles per GEMM half
SQRT2 = float(np.sqrt(2.0))

_cache = {}


def _emit_mm_group(nc, ps, xq, xs, w1, w2, n, interleave=None):
    """Emit one PSUM accumulation group for output tile (m, n).

    interleave: when emitting several m-groups k-by-k, the caller drives the
    k loop; this emits just one k's pair of matmuls.
    """
    ks = range(KD) if interleave is None else [interleave]
    for k in ks:
        ksl = slice(k * 128, (k + 1) * 128)
        nsl = slice(n * 512, (n + 1) * 512)
        nc.tensor.matmul(ps[:], xq[:, ksl], w1[k][:, nsl],
                         start=(k == 0), stop=False, skip_group_check=True)
        nc.tensor.matmul(ps[:], xs[:, ksl], w2[k][:, nsl],
                         start=False, stop=(k == KD - 1), skip_group_check=True)


def build_nc(reps=1):
    """Build + compile the per-core Bass program (cached per reps)."""
    key = ("nc", reps)
    if key in _cache:
        return _cache[key]

    nc = bacc.Bacc("TRN2", target_bir_lowering=False, debug=False)
    # x.T pre-tiled on host: xt[m, p, k*128 + j] = x[m*128+j, k*128+p]
    # -> per (m): SBUF tile [128, 1024] with fully-contiguous 4KB per partition
    xt_d = nc.dram_tensor("xt", [MT, 128, D], f32r, kind="ExternalInput").ap()
    mut_d = nc.dram_tensor("mut", [D, FL], f32, kind="ExternalInput").ap()
    sct_d = nc.dram_tensor("sct", [D, FL], f32, kind="ExternalInput").ap()
    out_d = nc.dram_tensor("out", [BL, FL], f32, kind="ExternalOutput").ap()

    with tile.TileContext(nc) as tc:
        with (
            nc.allow_low_precision(
                reason="f32r GEMM: ~12-bit mantissa is within the accuracy budget"
            ),
            tc.tile_pool(name="wpool", bufs=1) as wpool,
            tc.tile_pool(name="stage", bufs=3) as stage,
            tc.tile_pool(name="cpool", bufs=1) as cpool,
            tc.tile_pool(name="dram", bufs=1, space="DRAM") as dram,
            tc.tile_pool(name="xpool", bufs=4) as xpool,
            tc.tile_pool(name="opool", bufs=3) as opool,
            tc.tile_pool(name="pspool", bufs=6, space="PSUM") as pspool,
            tc.tile_pool(name="mmps", bufs=1, space="PSUM") as mmps_pool,
        ):
            for rep in range(reps):
                # ones column (f32r) for the partition-dim reduction of mm
                ones_t = cpool.tile([128, 1], f32, tag="ones")
                nc.gpsimd.memset(ones_t[:], 1.0)
                ones_r = cpool.tile([128, 1], f32r, tag="ones_r")
                nc.vector.tensor_copy(ones_r[:], ones_t[:])

                # ---- x strips + PSUM groups for m=0..1, emitted early ----
                NEARLY = 3
                xss, xqs, pss, ots = [], [], [], []
                for m in range(NEARLY):
                    xs = xpool.tile([128, D], f32r, tag="xs", name=f"xs{m}")
                    nc.sync.dma_start(xs[:], xt_d[m])
                    xq = xpool.tile([128, D], f32r, tag="xq", name=f"xq{m}")
                    nc.vector.tensor_mul(xq[:], xs[:].bitcast(f32), xs[:].bitcast(f32))
                    xss.append(xs)
                    xqs.append(xq)
                    pss.append([
                        pspool.tile([128, 512], f32, tag="ps", name=f"ps{m}_{n}")
                        for n in range(NT)
                    ])
                    ots.append(opool.tile([128, FL], f32, tag="ot", name=f"ot{m}"))
                mmps = [
                    mmps_pool.tile([1, 512], f32, tag=f"mmps{n}", name=f"mmps{n}")
                    for n in range(NT)
                ]

                # ---- W-prep interleaved with m=0..1 matmuls + mm reduction:
                # PE tracks the DVE k-by-k instead of idling behind the prologue
                w1 = {}
                w2 = {}
                for k in range(KD):
                    ksl = slice(k * 128, (k + 1) * 128)
                    mt_t = stage.tile([128, FL], f32, tag="mt")
                    nc.sync.dma_start(mt_t[:], mut_d[ksl, :])
                    st_t = stage.tile([128, FL], f32, tag="st")
                    nc.sync.dma_start(st_t[:], sct_d[ksl, :])
                    t2 = stage.tile([128, FL], f32, tag="t2")
                    nc.scalar.activation(t2[:], st_t[:], ACTF.Square)        # s^2
                    u = stage.tile([128, FL], f32, tag="u")
                    nc.scalar.activation(u[:], t2[:], ACTF.Copy, scale=-2.0)  # -2 s^2
                    w1t = wpool.tile([128, FL], f32r, tag=f"w1_{k}")
                    nc.vector.reciprocal(w1t[:], u[:])                       # -1/(2 s^2)
                    t3 = stage.tile([128, FL], f32, tag="t3")
                    nc.gpsimd.tensor_mul(t3[:], mt_t[:], w1t[:].bitcast(f32))  # mu*w1
                    w2t = wpool.tile([128, FL], f32r, tag=f"w2_{k}")
                    nc.scalar.activation(w2t[:], t3[:], ACTF.Copy, scale=-2.0)  # mu/s^2
                    m2it = stage.tile([128, FL], f32r, tag="m2i")
                    nc.vector.tensor_mul(m2it[:], mt_t[:], t3[:])            # mu^2*w1
                    w1[k] = w1t
                    w2[k] = w2t

                    for m in range(NEARLY):
                        for n in range(NT):
                            _emit_mm_group(nc, pss[m][n], xqs[m], xss[m],
                                           w1, w2, n, interleave=k)
                    for n in range(NT):
                        nsl = slice(n * 512, (n + 1) * 512)
                        nc.tensor.matmul(mmps[n][:], ones_r[:], m2it[:, nsl],
                                         start=(k == 0), stop=(k == KD - 1),
                                         skip_group_check=True)

                # Early groups: evacuate with a plain copy so the PSUM banks
                # free immediately instead of waiting for the mm broadcast.
                for m in range(NEARLY):
                    for n in range(NT):
                        nc.vector.tensor_copy(
                            ots[m][:, n * 512:(n + 1) * 512], pss[m][n][:]
                        )

                mmsb = cpool.tile([1, FL], f32, tag="mmsb")
                for n in range(NT):
                    nc.vector.tensor_copy(mmsb[:, n * 512:(n + 1) * 512], mmps[n][:])
                mm_dram = dram.tile([1, FL], f32, name=f"mmd{rep}")
                nc.sync.dma_start(mm_dram[:], mmsb[:])
                mmbc = cpool.tile([128, FL], f32, tag="mmbc")
                nc.sync.dma_start(mmbc[:], mm_dram[:].to_broadcast((128, FL)))

                def finish(m, ps_n, ot):
                    for n in range(NT):
                        nc.vector.tensor_add(
                            ot[:, n * 512:(n + 1) * 512], ps_n[n][:],
                            mmbc[:, n * 512:(n + 1) * 512],
                        )
                    nc.sync.dma_start(out_d[m * 128:(m + 1) * 128, :], ot[:])

                for m in range(NEARLY):
                    nc.vector.tensor_add(ots[m][:], ots[m][:], mmbc[:])
                    nc.sync.dma_start(out_d[m * 128:(m + 1) * 128, :], ots[m][:])

                # ---- steady-state main loop ----
                for m in range(NEARLY, MT):
                    xs = xpool.tile([128, D], f32r, tag="xs", name=f"xs{m}")
                    nc.sync.dma_start(xs[:], xt_d[m])
                    xq = xpool.tile([128, D], f32r, tag="xq", name=f"xq{m}")
                    nc.vector.tensor_mul(xq[:], xs[:].bitcast(f32), xs[:].bitcast(f32))
                    ot = opool.tile([128, FL], f32, tag="ot", name=f"ot{m}")
                    ps_n = []
                    for n in range(NT):
                        ps = pspool.tile([128, 512], f32, tag="ps", name=f"ps{m}_{n}")
                        _emit_mm_group(nc, ps, xq, xs, w1, w2, n)
                        ps_n.append(ps)
                    finish(m, ps_n, ot)

    nc.compile()
    _cache[key] = nc
    return nc


def make_in_maps(x, mu, scale_diag):
    """Host-side shard + layout prep (free: not on the measured HW path)."""
    x = np.ascontiguousarray(x, dtype=np.float32)
    mu = np.ascontiguousarray(mu, dtype=np.float32)
    scale_diag = np.ascontiguousarray(scale_diag, dtype=np.float32)

    in_maps = []
    for c in range(NB * NF):
        ib, jf = divmod(c, NF)
        xsl = x[ib * BL:(ib + 1) * BL]  # [4096, 1024]
        # xt[m, p, k*128+j] = xsl[m*128+j, k*128+p]
        xt = np.ascontiguousarray(
            xsl.reshape(MT, 128, KD, 128).transpose(0, 3, 2, 1).reshape(MT, 128, D)
        )
        musl = mu[jf * FL:(jf + 1) * FL]        # [1024, 1024]
        scsl = scale_diag[jf * FL:(jf + 1) * FL]
        in_maps.append({
            "xt": xt,
            "mut": np.ascontiguousarray(musl.T),
            "sct": np.ascontiguousarray(scsl.T),
        })
    return in_maps


def gather(results):
    out = np.empty((B, F), dtype=np.float32)
    for c in range(NB * NF):
        ib, jf = divmod(c, NF)
        out[ib * BL:(ib + 1) * BL, jf * FL:(jf + 1) * FL] = results[c]["out"]
    return out


def kernel(x, mu, scale_diag):
    nc = build_nc()
    in_maps = make_in_maps(x, mu, scale_diag)
    r = bass_utils.run_bass_kernel_spmd(nc, in_maps, core_ids=list(range(NB * NF)))
    return gather(r.results)


if __name__ == "__main__":
    rng = np.random.default_rng(0)
    x = rng.standard_normal((B, D), dtype=np.float32)
    mu = rng.standard_normal((F, D), dtype=np.float32)
    sc = rng.uniform(0.5, 1.5, size=(F, D)).astype(np.float32)
    got = kernel(x, mu, sc)
    inv2 = 1.0 / (sc.astype(np.float64) ** 2)
    xx = (x.astype(np.float64) ** 2) @ inv2.T
    xm = x.astype(np.float64) @ (mu * inv2).T
    mm = (mu.astype(np.float64) ** 2 * inv2).sum(-1)
    want = -0.5 * (xx - 2 * xm + mm[None, :])
    err = np.abs(got - want).max() / np.abs(want).max()
    print("rel err vs fp64:", err)


# revision 14
# speedup vs baseline: 81.4351x; 1.4354x over previous
"""Trainium2 Bass kernel for nn_DenseSOFLayer (diag-Gaussian log-prob, GEMM form).

out[b, f] = -0.5 * sum_d ((x[b,d] - mu[f,d]) / scale[f,d])^2
          = sum_d x^2[b,d] * w1[f,d] + x[b,d] * w2[f,d] + mm[f]
  w1 = -1/(2 s^2), w2 = mu/s^2, mm[f] = sum_d mu^2 * w1

Sharding: 2 (batch) x 4 (feature) grid over 8 cores.
Per core: C_local[4096, 1024] = A[4096, 2048] @ W[2048, 1024] via f32r matmuls
(full-rate PE, ~12-bit mantissa) with the mm row folded in during PSUM
evacuation through a DMA partition-broadcast.

Emission order is tuned so the PE starts the m=0/m=1 output tiles while the
W-prep (ACT+DVE) is still streaming k-tiles, instead of idling behind the
whole prologue.
"""

import sys

if "/opt/trn_rl_repo" not in sys.path:
    sys.path.insert(0, "/opt/trn_rl_repo")

import numpy as np

import concourse.bass as bass
import concourse.mybir as mybir
import concourse.tile as tile
from concourse import bacc, bass_utils

f32 = mybir.dt.float32
f32r = mybir.dt.float32r
ACTF = mybir.ActivationFunctionType

B, F, D = 8192, 4096, 1024
NB, NF = 2, 4              # core grid: batch-split x feature-split
BL, FL = B // NB, F // NF  # 4096, 1024 per core
MT = BL // 128             # 32 m-tiles
NT = FL // 512             # 2 n-tiles
KD = D // 128              # 8 contraction tiles per GEMM half
SQRT2 = float(np.sqrt(2.0))

_cache = {}

# structural experiment flags (validated via TimelineSim + HW A/B)
# n-interleave won its reps=8 hardware A/B by ~5.6 us/body (4 of 6 rounds,
# median of medians); store_gp/xq_gp were sim-neutral and not adopted.
OPT_INTERLEAVE_N = True    # alternate PSUM banks between consecutive matmuls
OPT_STORE_GP = False       # issue output stores on the SWDGE (gpsimd) queue
OPT_XQ_GP = False          # square x on gpsimd instead of DVE in steady loop


def _emit_mm_group(nc, ps, xq, xs, w1, w2, n, interleave=None):
    """Emit one PSUM accumulation group for output tile (m, n).

    interleave: when emitting several m-groups k-by-k, the caller drives the
    k loop; this emits just one k's pair of matmuls.
    """
    ks = range(KD) if interleave is None else [interleave]
    for k in ks:
        ksl = slice(k * 128, (k + 1) * 128)
        nsl = slice(n * 512, (n + 1) * 512)
        nc.tensor.matmul(ps[:], xq[:, ksl], w1[k][:, nsl],
                         start=(k == 0), stop=False, skip_group_check=True)
        nc.tensor.matmul(ps[:], xs[:, ksl], w2[k][:, nsl],
                         start=False, stop=(k == KD - 1), skip_group_check=True)


def _emit_mm_groups_ninterleaved(nc, ps_n, xq, xs, w1, w2):
    """Both n-groups for one m, consecutive matmuls alternating PSUM banks."""
    for k in range(KD):
        ksl = slice(k * 128, (k + 1) * 128)
        for n in range(NT):
            nsl = slice(n * 512, (n + 1) * 512)
            nc.tensor.matmul(ps_n[n][:], xq[:, ksl], w1[k][:, nsl],
                             start=(k == 0), stop=False, skip_group_check=True)
        for n in range(NT):
            nsl = slice(n * 512, (n + 1) * 512)
            nc.tensor.matmul(ps_n[n][:], xs[:, ksl], w2[k][:, nsl],
                             start=False, stop=(k == KD - 1),
                             skip_group_check=True)


def build_nc(reps=1):
    """Build + compile the per-core Bass program (cached per reps)."""
    key = ("nc", reps)
    if key in _cache:
        return _cache[key]

    nc = bacc.Bacc("TRN2", target_bir_lowering=False, debug=False)
    # x.T pre-tiled on host: xt[m, p, k*128 + j] = x[m*128+j, k*128+p]
    # -> per (m): SBUF tile [128, 1024] with fully-contiguous 4KB per partition
    xt_d = nc.dram_tensor("xt", [MT, 128, D], f32r, kind="ExternalInput").ap()
    mut_d = nc.dram_tensor("mut", [D, FL], f32, kind="ExternalInput").ap()
    sct_d = nc.dram_tensor("sct", [D, FL], f32, kind="ExternalInput").ap()
    out_d = nc.dram_tensor("out", [BL, FL], f32, kind="ExternalOutput").ap()

    with tile.TileContext(nc) as tc:
        with (
            nc.allow_low_precision(
                reason="f32r GEMM: ~12-bit mantissa is within the accuracy budget"
            ),
            tc.tile_pool(name="wpool", bufs=1) as wpool,
            tc.tile_pool(name="stage", bufs=3) as stage,
            tc.tile_pool(name="cpool", bufs=1) as cpool,
            tc.tile_pool(name="dram", bufs=1, space="DRAM") as dram,
            tc.tile_pool(name="xpool", bufs=4) as xpool,
            tc.tile_pool(name="opool", bufs=3) as opool,
            tc.tile_pool(name="pspool", bufs=6, space="PSUM") as pspool,
            tc.tile_pool(name="mmps", bufs=1, space="PSUM") as mmps_pool,
        ):
            for rep in range(reps):
                # ones column (f32r) for the partition-dim reduction of mm
                ones_t = cpool.tile([128, 1], f32, tag="ones")
                nc.gpsimd.memset(ones_t[:], 1.0)
                ones_r = cpool.tile([128, 1], f32r, tag="ones_r")
                nc.vector.tensor_copy(ones_r[:], ones_t[:])

                # ---- x strips + PSUM groups for m=0..1, emitted early ----
                NEARLY = 3
                xss, xqs, pss, ots = [], [], [], []
                for m in range(NEARLY):
                    xs = xpool.tile([128, D], f32r, tag="xs", name=f"xs{m}")
                    nc.sync.dma_start(xs[:], xt_d[m])
                    xq = xpool.tile([128, D], f32r, tag="xq", name=f"xq{m}")
                    nc.vector.tensor_mul(xq[:], xs[:].bitcast(f32), xs[:].bitcast(f32))
                    xss.append(xs)
                    xqs.append(xq)
                    pss.append([
                        pspool.tile([128, 512], f32, tag="ps", name=f"ps{m}_{n}")
                        for n in range(NT)
                    ])
                    ots.append(opool.tile([128, FL], f32, tag="ot", name=f"ot{m}"))
                mmps = [
                    mmps_pool.tile([1, 512], f32, tag=f"mmps{n}", name=f"mmps{n}")
                    for n in range(NT)
                ]

                # ---- W-prep interleaved with m=0..1 matmuls + mm reduction:
                # PE tracks the DVE k-by-k instead of idling behind the prologue
                w1 = {}
                w2 = {}
                for k in range(KD):
                    ksl = slice(k * 128, (k + 1) * 128)
                    mt_t = stage.tile([128, FL], f32, tag="mt")
                    nc.sync.dma_start(mt_t[:], mut_d[ksl, :])
                    st_t = stage.tile([128, FL], f32, tag="st")
                    nc.sync.dma_start(st_t[:], sct_d[ksl, :])
                    t2 = stage.tile([128, FL], f32, tag="t2")
                    nc.scalar.activation(t2[:], st_t[:], ACTF.Square)        # s^2
                    u = stage.tile([128, FL], f32, tag="u")
                    nc.scalar.activation(u[:], t2[:], ACTF.Copy, scale=-2.0)  # -2 s^2
                    w1t = wpool.tile([128, FL], f32r, tag=f"w1_{k}")
                    nc.vector.reciprocal(w1t[:], u[:])                       # -1/(2 s^2)
                    t3 = stage.tile([128, FL], f32, tag="t3")
                    nc.gpsimd.tensor_mul(t3[:], mt_t[:], w1t[:].bitcast(f32))  # mu*w1
                    w2t = wpool.tile([128, FL], f32r, tag=f"w2_{k}")
                    nc.scalar.activation(w2t[:], t3[:], ACTF.Copy, scale=-2.0)  # mu/s^2
                    m2it = stage.tile([128, FL], f32r, tag="m2i")
                    nc.vector.tensor_mul(m2it[:], mt_t[:], t3[:])            # mu^2*w1
                    w1[k] = w1t
                    w2[k] = w2t

                    for m in range(NEARLY):
                        for n in range(NT):
                            _emit_mm_group(nc, pss[m][n], xqs[m], xss[m],
                                           w1, w2, n, interleave=k)
                    for n in range(NT):
                        nsl = slice(n * 512, (n + 1) * 512)
                        nc.tensor.matmul(mmps[n][:], ones_r[:], m2it[:, nsl],
                                         start=(k == 0), stop=(k == KD - 1),
                                         skip_group_check=True)

                # Early groups: evacuate with a plain copy so the PSUM banks
                # free immediately instead of waiting for the mm broadcast.
                for m in range(NEARLY):
                    for n in range(NT):
                        nc.vector.tensor_copy(
                            ots[m][:, n * 512:(n + 1) * 512], pss[m][n][:]
                        )

                mmsb = cpool.tile([1, FL], f32, tag="mmsb")
                for n in range(NT):
                    nc.vector.tensor_copy(mmsb[:, n * 512:(n + 1) * 512], mmps[n][:])
                mm_dram = dram.tile([1, FL], f32, name=f"mmd{rep}")
                nc.sync.dma_start(mm_dram[:], mmsb[:])
                mmbc = cpool.tile([128, FL], f32, tag="mmbc")
                nc.sync.dma_start(mmbc[:], mm_dram[:].to_broadcast((128, FL)))

                store_eng = nc.gpsimd if OPT_STORE_GP else nc.sync

                def finish(m, ps_n, ot):
                    for n in range(NT):
                        nc.vector.tensor_add(
                            ot[:, n * 512:(n + 1) * 512], ps_n[n][:],
                            mmbc[:, n * 512:(n + 1) * 512],
                        )
                    store_eng.dma_start(out_d[m * 128:(m + 1) * 128, :], ot[:])

                for m in range(NEARLY):
                    nc.vector.tensor_add(ots[m][:], ots[m][:], mmbc[:])
                    store_eng.dma_start(out_d[m * 128:(m + 1) * 128, :], ots[m][:])

                # ---- steady-state main loop ----
                for m in range(NEARLY, MT):
                    xs = xpool.tile([128, D], f32r, tag="xs", name=f"xs{m}")
                    nc.sync.dma_start(xs[:], xt_d[m])
                    xq = xpool.tile([128, D], f32r, tag="xq", name=f"xq{m}")
                    sq_eng = nc.gpsimd if OPT_XQ_GP else nc.vector
                    sq_eng.tensor_mul(xq[:], xs[:].bitcast(f32), xs[:].bitcast(f32))
                    ot = opool.tile([128, FL], f32, tag="ot", name=f"ot{m}")
                    ps_n = [
                        pspool.tile([128, 512], f32, tag="ps", name=f"ps{m}_{n}")
                        for n in range(NT)
                    ]
                    if OPT_INTERLEAVE_N:
                        _emit_mm_groups_ninterleaved(nc, ps_n, xq, xs, w1, w2)
                    else:
                        for n in range(NT):
                            _emit_mm_group(nc, ps_n[n], xq, xs, w1, w2, n)
                    finish(m, ps_n, ot)

    nc.compile()
    _cache[key] = nc
    return nc


def make_in_maps(x, mu, scale_diag):
    """Host-side shard + layout prep (free: not on the measured HW path)."""
    x = np.ascontiguousarray(x, dtype=np.float32)
    mu = np.ascontiguousarray(mu, dtype=np.float32)
    scale_diag = np.ascontiguousarray(scale_diag, dtype=np.float32)

    in_maps = []
    for c in range(NB * NF):
        ib, jf = divmod(c, NF)
        xsl = x[ib * BL:(ib + 1) * BL]  # [4096, 1024]
        # xt[m, p, k*128+j] = xsl[m*128+j, k*128+p]
        xt = np.ascontiguousarray(
            xsl.reshape(MT, 128, KD, 128).transpose(0, 3, 2, 1).reshape(MT, 128, D)
        )
        musl = mu[jf * FL:(jf + 1) * FL]        # [1024, 1024]
        scsl = scale_diag[jf * FL:(jf + 1) * FL]
        in_maps.append({
            "xt": xt,
            "mut": np.ascontiguousarray(musl.T),
            "sct": np.ascontiguousarray(scsl.T),
        })
    return in_maps


def gather(results):
    out = np.empty((B, F), dtype=np.float32)
    for c in range(NB * NF):
        ib, jf = divmod(c, NF)
        out[ib * BL:(ib + 1) * BL, jf * FL:(jf + 1) * FL] = results[c]["out"]
    return out


def kernel(x, mu, scale_diag):
    nc = build_nc()
    in_maps = make_in_maps(x, mu, scale_diag)
    r = bass_utils.run_bass_kernel_spmd(nc, in_maps, core_ids=list(range(NB * NF)))
    return gather(r.results)


if __name__ == "__main__":
    rng = np.random.default_rng(0)
    x = rng.standard_normal((B, D), dtype=np.float32)
    mu = rng.standard_normal((F, D), dtype=np.float32)
    sc = rng.uniform(0.5, 1.5, size=(F, D)).astype(np.float32)
    got = kernel(x, mu, sc)
    inv2 = 1.0 / (sc.astype(np.float64) ** 2)
    xx = (x.astype(np.float64) ** 2) @ inv2.T
    xm = x.astype(np.float64) @ (mu * inv2).T
    mm = (mu.astype(np.float64) ** 2 * inv2).sum(-1)
    want = -0.5 * (xx - 2 * xm + mm[None, :])
    err = np.abs(got - want).max() / np.abs(want).max()
    print("rel err vs fp64:", err)
